# revision 1
# baseline (speedup 1.0000x reference)
"""Distributed Trainium2 kernel for nn_Contrast_loss (row-parallel InfoNCE).

Math (reference):
  h1 = proj(pri), h2 = proj(aux)   with proj(z) = elu(z@W1.T+b1)@W2.T+b2
  n1 = normalize(h1), n2 = normalize(h2)
  l1_i = log(den1_i) - 2*d12_i,  den1_i = sum_j e^{2 S11_ij} + sum_j e^{2 S12_ij} - e^{2 S11_ii}
  l2_i = log(den2_i) - 2*d12_i,  den2_i = sum_j e^{2 S22_ij} + sum_j e^{2 S12_ji} - e^{2 S22_ii}
  loss = mean((l1+l2)/2)
  (S11_ii = S22_ii = 1 since rows are unit-normalized; d12_i = n1_i . n2_i)

Sharding: rows split across 8 cores (1024 rows each). Each core projects +
normalizes its row block (transposed layout [D, rows]), AllGathers n1T/n2T,
then computes its row-block of S11/S12/S22 with fp32r matmuls. exp(2x) and
row sums are fused on the scalar engine (accum_out); S12 column partials are
accumulated on the vector engine and partition-reduced with ones-matmuls.
Per-core partial sums are assembled into the scalar loss on CPU (O(N) work).

fp32r notes: operands reaching an fp32r matmul must hold values rounded to
the bf16-pair representable set — raw fp32 via DMA faults the exec unit.
External inputs are pre-rounded on CPU; on-device tensors feeding matmuls
are produced by DVE ops with float32r output dtype (engine rounds on write).
"""

import os
import numpy as np
import ml_dtypes

import concourse.bass as bass
import concourse.tile as tile
from concourse import mybir, bacc
from concourse.bass_utils import run_bass_kernel_spmd

NCORES = 8
N = 8192
D = 512
R = N // NCORES          # rows per core = 1024
KC = D // 128            # contraction chunks = 4
MT = R // 128            # row tiles per core = 8
BB = 4                   # column super-blocks (each = 2048 cols = 2 source cores)
F32 = mybir.dt.float32
F32R = mybir.dt.float32r

_CACHE = {}


def _build():
    STAGE = int(os.environ.get("BASS_STAGE", "4"))
    nc = bacc.Bacc("TRN2", target_bir_lowering=False, debug=False,
                   num_devices=NCORES)

    z1t = nc.dram_tensor("z1t", [D, R], F32R, kind="ExternalInput")
    z2t = nc.dram_tensor("z2t", [D, R], F32R, kind="ExternalInput")
    w1t = nc.dram_tensor("w1t", [D, D], F32R, kind="ExternalInput")
    w2t = nc.dram_tensor("w2t", [D, D], F32R, kind="ExternalInput")
    b1c = nc.dram_tensor("b1c", [128, KC], F32, kind="ExternalInput")
    b2c = nc.dram_tensor("b2c", [128, KC], F32, kind="ExternalInput")

    rs_out = nc.dram_tensor("rs", [128, 3 * MT * BB], F32, kind="ExternalOutput")
    cs_out = nc.dram_tensor("colsum", [16, 512], F32, kind="ExternalOutput")
    d12_out = nc.dram_tensor("d12", [2, 512], F32, kind="ExternalOutput")

    n_all = nc.dram_tensor("n_all", [NCORES, 2, KC, 128, R], F32R,
                           addr_space="Shared")

    EXP = mybir.ActivationFunctionType.Exp

    with tile.TileContext(nc) as tc:
        with tc.tile_pool(name="keep", bufs=1) as kp, \
             tc.tile_pool(name="dr", bufs=1, space="DRAM") as dr:

            # ---- persistent tiles ----
            b1s = kp.tile([128, KC], F32, name="b1s", tag="b1s")
            b2s = kp.tile([128, KC], F32, name="b2s", tag="b2s")
            nc.sync.dma_start(out=b1s, in_=b1c[:, :])
            nc.sync.dma_start(out=b2s, in_=b2c[:, :])
            ones_k = kp.tile([128, 1], F32, name="ones_k", tag="ones_k")
            nc.vector.memset(ones_k, 1.0)
            rs = kp.tile([128, 3 * MT * BB], F32, name="rs", tag="rs")
            nc.vector.memset(rs, 0.0)
            nt = [[kp.tile([128, R], F32R, name=f"nt{e}_{k}", tag=f"nt{e}_{k}")
                   for k in range(KC)] for e in range(2)]
            n_loc = dr.tile([2, KC, 128, R], F32R, name="n_loc", tag="n_loc")

            # ---- projection + normalize (scoped pool) ----
            with tc.tile_pool(name="proj", bufs=1) as pj, \
                 tc.tile_pool(name="psp", bufs=1, space="PSUM") as psp:
                w1 = [pj.tile([128, D], F32R, name=f"w1_{k}", tag=f"w1_{k}")
                      for k in range(KC)]
                w2 = [pj.tile([128, D], F32R, name=f"w2_{k}", tag=f"w2_{k}")
                      for k in range(KC)]
                for k in range(KC):
                    nc.sync.dma_start(out=w1[k], in_=w1t[k * 128:(k + 1) * 128, :])
                    nc.sync.dma_start(out=w2[k], in_=w2t[k * 128:(k + 1) * 128, :])
                ones_b = pj.tile([1, 128], F32, name="ones_b", tag="ones_b")
                nc.vector.memset(ones_b, 1.0)

                for e, zdram in enumerate((z1t, z2t)):
                    zt = [pj.tile([128, R], F32R, name=f"zt_{k}", tag=f"zt_{k}")
                          for k in range(KC)]
                    for k in range(KC):
                        nc.sync.dma_start(out=zt[k],
                                          in_=zdram[k * 128:(k + 1) * 128, :])

                    # layer 1 + elu
                    et = [pj.tile([128, R], F32R, name=f"et_{k}", tag=f"et_{k}")
                          for k in range(KC)]
                    for oc in range(KC):
                        pa = psp.tile([128, R], F32, name="pa", tag="pa", bufs=2)
                        for h in range(R // 512):
                            for k in range(KC):
                                nc.tensor.matmul(
                                    pa[:, h * 512:(h + 1) * 512],
                                    w1[k][:, oc * 128:(oc + 1) * 128],
                                    zt[k][:, h * 512:(h + 1) * 512],
                                    start=(k == 0), stop=(k == KC - 1))
                        t1 = pj.tile([128, R], F32, name="t1", tag="t1")
                        t2 = pj.tile([128, R], F32, name="t2", tag="t2")
                        nc.scalar.activation(t1, pa, EXP, bias=b1s[:, oc:oc + 1])
                        nc.vector.tensor_scalar_sub(t1, t1, 1.0)
                        nc.scalar.activation(t2, pa,
                                             mybir.ActivationFunctionType.Relu,
                                             bias=b1s[:, oc:oc + 1])
                        nc.vector.tensor_tensor(et[oc], t1, t2,
                                                mybir.AluOpType.min)

                    # layer 2 + bias; squared norms
                    ht = [pj.tile([128, R], F32, name=f"ht_{k}", tag=f"ht_{k}")
                          for k in range(KC)]
                    nsq = pj.tile([128, R], F32, name="nsq", tag="nsq")
                    for pc in range(KC):
                        ph = psp.tile([128, R], F32, name="pa", tag="pa", bufs=2)
                        for h in range(R // 512):
                            for k in range(KC):
                                nc.tensor.matmul(
                                    ph[:, h * 512:(h + 1) * 512],
                                    w2[k][:, pc * 128:(pc + 1) * 128],
                                    et[k][:, h * 512:(h + 1) * 512],
                                    start=(k == 0), stop=(k == KC - 1))
                        nc.scalar.activation(ht[pc], ph,
                                             mybir.ActivationFunctionType.Identity,
                                             bias=b2s[:, pc:pc + 1])
                        if pc == 0:
                            nc.vector.tensor_mul(nsq, ht[pc], ht[pc])
                        else:
                            sq = pj.tile([128, R], F32, name="t1", tag="t1")
                            nc.vector.tensor_mul(sq, ht[pc], ht[pc])
                            nc.vector.tensor_add(nsq, nsq, sq)

                    # 1/norm, broadcast, normalize
                    nrm = psp.tile([1, R], F32, name="nrm", tag="nrm", bufs=1)
                    for h in range(R // 512):
                        nc.tensor.matmul(nrm[0:1, h * 512:(h + 1) * 512],
                                         ones_k,
                                         nsq[:, h * 512:(h + 1) * 512],
                                         start=True, stop=True)
                    sr = pj.tile([1, R], F32, name="sr", tag="sr")
                    nc.scalar.activation(sr, nrm,
                                         mybir.ActivationFunctionType.Sqrt)
                    nc.vector.reciprocal(sr, sr)
                    bc = psp.tile([128, R], F32, name="bc", tag="bc", bufs=1)
                    for h in range(R // 512):
                        nc.tensor.matmul(bc[:, h * 512:(h + 1) * 512],
                                         ones_b,
                                         sr[0:1, h * 512:(h + 1) * 512],
                                         start=True, stop=True)
                    for pc in range(KC):
                        nc.vector.tensor_mul(nt[e][pc], ht[pc], bc)
                        nc.sync.dma_start(out=n_loc[e, pc], in_=nt[e][pc])

            # ---- all-gather both normalized matrices ----
            if STAGE >= 2:
                nc.gpsimd.collective_compute(
                    "AllGather", mybir.AluOpType.bypass,
                    replica_groups=[list(range(NCORES))],
                    ins=[n_loc[:].opt()],
                    outs=[n_all[:].opt()])

            # ---- main similarity loops (scoped pool) ----
            with tc.tile_pool(name="main", bufs=1) as mn:
              with tc.tile_pool(name="psm", bufs=1, space="PSUM") as psm:
                acc = mn.tile([128, N], F32, name="acc", tag="acc")
                nc.vector.memset(acc, 0.0)

                # d12 row-dot products (independent of the gather)
                mp = mn.tile([128, R], F32, name="mp", tag="mp")
                m2 = mn.tile([128, R], F32, name="m2", tag="m2")
                nc.vector.tensor_mul(mp, nt[0][0], nt[1][0])
                for k in range(1, KC):
                    nc.vector.tensor_mul(m2, nt[0][k], nt[1][k])
                    nc.vector.tensor_add(mp, mp, m2)

                def mm_group(pg, own, res, m):
                    for t in range(4):
                        j, half = t // 2, t % 2
                        for k in range(KC):
                            nc.tensor.matmul(
                                pg[:, t * 512:(t + 1) * 512],
                                own[k][:, m * 128:(m + 1) * 128],
                                res[j][k][:, half * 512:(half + 1) * 512],
                                start=(k == 0), stop=(k == KC - 1))

                # phase B: S11 (rhs = gathered n1T)
                for bb in (range(BB) if STAGE >= 2 else []):
                    res = [[mn.tile([128, R], F32R, name=f"r{j}_{k}",
                                    tag=f"r{j}_{k}", bufs=2)
                            for k in range(KC)] for j in range(2)]
                    for j in range(2):
                        for k in range(KC):
                            nc.sync.dma_start(out=res[j][k],
                                              in_=n_all[2 * bb + j, 0, k])
                    for m in range(MT):
                        pg = psm.tile([128, 2048], F32, name="pg", tag="pg",
                                      bufs=2)
                        mm_group(pg, nt[0], res, m)
                        col = 0 * (MT * BB) + m * BB + bb
                        nc.scalar.activation(pg, pg, EXP, scale=2.0,
                                             accum_out=rs[:, col:col + 1])

                # phase CD: S12 + S22 (rhs = gathered n2T)
                for bb in (range(BB) if STAGE >= 3 else []):
                    res = [[mn.tile([128, R], F32R, name=f"r{j}_{k}",
                                    tag=f"r{j}_{k}", bufs=2)
                            for k in range(KC)] for j in range(2)]
                    for j in range(2):
                        for k in range(KC):
                            nc.sync.dma_start(out=res[j][k],
                                              in_=n_all[2 * bb + j, 1, k])
                    for m in range(MT):
                        pg = psm.tile([128, 2048], F32, name="pg", tag="pg",
                                      bufs=2)
                        mm_group(pg, nt[0], res, m)
                        col = 1 * (MT * BB) + m * BB + bb
                        nc.scalar.activation(pg, pg, EXP, scale=2.0,
                                             accum_out=rs[:, col:col + 1])
                        nc.vector.tensor_add(acc[:, bb * 2048:(bb + 1) * 2048],
                                             acc[:, bb * 2048:(bb + 1) * 2048],
                                             pg)

                        pg2 = psm.tile([128, 2048], F32, name="pg", tag="pg",
                                       bufs=2)
                        mm_group(pg2, nt[1], res, m)
                        col = 2 * (MT * BB) + m * BB + bb
                        nc.scalar.activation(pg2, pg2, EXP, scale=2.0,
                                             accum_out=rs[:, col:col + 1])

              # ---- tails: partition reductions via ones-matmuls ----
              with tc.tile_pool(name="pst", bufs=1, space="PSUM") as pst:
                if True:
                    for h in range(2):
                        dp = pst.tile([1, 512], F32, name="dp", tag="dp", bufs=2)
                        nc.tensor.matmul(dp, ones_k,
                                         mp[:, h * 512:(h + 1) * 512],
                                         start=True, stop=True)
                        stg = mn.tile([1, 512], F32, name="stg", tag="stg",
                                      bufs=4)
                        nc.vector.tensor_copy(stg, dp)
                        nc.sync.dma_start(out=d12_out[h:h + 1, :], in_=stg)
                    for j in range(16):
                        cp = pst.tile([1, 512], F32, name="cp", tag="cp", bufs=4)
                        nc.tensor.matmul(cp, ones_k,
                                         acc[:, j * 512:(j + 1) * 512],
                                         start=True, stop=True)
                        stg = mn.tile([1, 512], F32, name="stg", tag="stg",
                                      bufs=4)
                        nc.vector.tensor_copy(stg, cp)
                        nc.sync.dma_start(out=cs_out[j:j + 1, :], in_=stg)
            nc.sync.dma_start(out=rs_out[:, :], in_=rs)

    nc.compile()
    return nc


def _get_nc():
    if "nc" not in _CACHE:
        _CACHE["nc"] = _build()
    return _CACHE["nc"]


def _round_f32r(a):
    """round to the bf16-pair representable set required by fp32r matmuls"""
    hi = a.astype(ml_dtypes.bfloat16).astype(np.float32)
    lo = (a - hi).astype(ml_dtypes.bfloat16).astype(np.float32)
    return hi + lo


def make_in_maps(pri, aux, W1, b1, W2, b2):
    pri = np.asarray(pri, dtype=np.float32)
    aux = np.asarray(aux, dtype=np.float32)
    w1t = _round_f32r(np.ascontiguousarray(np.asarray(W1, dtype=np.float32).T))
    w2t = _round_f32r(np.ascontiguousarray(np.asarray(W2, dtype=np.float32).T))
    b1 = np.asarray(b1, dtype=np.float32)
    b2 = np.asarray(b2, dtype=np.float32)
    b1c = np.ascontiguousarray(b1.reshape(KC, 128).T)
    b2c = np.ascontiguousarray(b2.reshape(KC, 128).T)
    priT = _round_f32r(np.ascontiguousarray(pri.T))
    auxT = _round_f32r(np.ascontiguousarray(aux.T))

    in_maps = []
    for c in range(NCORES):
        sl = slice(c * R, (c + 1) * R)
        in_maps.append({
            "z1t": np.ascontiguousarray(priT[:, sl]),
            "z2t": np.ascontiguousarray(auxT[:, sl]),
            "w1t": w1t, "w2t": w2t, "b1c": b1c, "b2c": b2c,
        })
    return in_maps


def assemble(results):
    """CPU assembly of the scalar loss from per-core partials"""
    E2 = np.exp(np.float64(2.0))
    colsum_full = np.zeros(N, dtype=np.float64)
    for c in range(NCORES):
        colsum_full += results[c]["colsum"].reshape(N).astype(np.float64)

    total = np.float64(0.0)
    for c in range(NCORES):
        rs = results[c]["rs"].astype(np.float64)      # [128, 96]
        r = rs.reshape(128, 3, MT, BB).sum(-1)        # [128, 3, MT]
        # row i_local = m*128 + p  ->  transpose to [MT, 128] then flatten
        rs11 = r[:, 0, :].T.reshape(R)
        rs12 = r[:, 1, :].T.reshape(R)
        rs22 = r[:, 2, :].T.reshape(R)
        d12 = results[c]["d12"].astype(np.float64).reshape(R)
        den1 = rs11 + rs12 - E2
        den2 = rs22 + colsum_full[c * R:(c + 1) * R] - E2
        li = 0.5 * (np.log(den1) + np.log(den2)) - 2.0 * d12
        total += li.sum()

    return np.float32(total / N)


def kernel(pri_embedding, aux_embedding, W1, b1, W2, b2):
    in_maps = make_in_maps(pri_embedding, aux_embedding, W1, b1, W2, b2)
    nc = _get_nc()
    res = run_bass_kernel_spmd(nc, in_maps, list(range(NCORES))).results
    return assemble(res)



# revision 9
# speedup vs baseline: 1.5884x; 1.5884x over previous
"""Distributed Trainium2 kernel for nn_Contrast_loss (row-parallel InfoNCE).

Math (reference):
  h1 = proj(pri), h2 = proj(aux)   with proj(z) = elu(z@W1.T+b1)@W2.T+b2
  n1 = normalize(h1), n2 = normalize(h2)
  l1_i = log(den1_i) - 2*d12_i,  den1_i = sum_j e^{2 S11_ij} + sum_j e^{2 S12_ij} - e^{2 S11_ii}
  l2_i = log(den2_i) - 2*d12_i,  den2_i = sum_j e^{2 S22_ij} + sum_j e^{2 S12_ji} - e^{2 S22_ii}
  loss = mean((l1+l2)/2)

Sharding: rows split across 8 cores (1024 rows each). Each core projects +
normalizes its row block in fp32 (transposed layout [D, rows]), computes d12
from the fp32 values, then quantizes the normalized rows to fp8e4 and
AllGathers them (one collective per embedding so the first gather overlaps
the second projection). The three NxN similarity matrices are computed in
fp8 DoubleRow matmuls (2x bf16 rate); exp(2x) row sums are fused on the
scalar engine (accum_out); S12 column partials accumulate on the vector
engine and are partition-reduced with ones-matmuls. While the second gather
is in flight each core computes its own local S11 block (self x self) to
keep the PE array warm; those row sums are duplicates and are ignored by
the CPU assembly. Per-core partials are assembled into the scalar loss on
CPU (O(N) work).

fp8 numerics: quantizing the normalized rows to e4m3 perturbs each S entry
by ~1e-3 absolute; the perturbations average out in the 16k-term exp sums
(measured end-to-end loss rel err ~1e-5, gate is 2e-2). d12 enters the loss
linearly and is kept in fp32.
"""

import numpy as np
import ml_dtypes

import concourse.bass as bass
import concourse.tile as tile
from concourse import mybir, bacc
from concourse.bass_utils import run_bass_kernel_spmd

NCORES = 8
N = 8192
D = 512
R = N // NCORES          # rows per core = 1024
KC = D // 128            # contraction chunks = 4
MT = R // 128            # row tiles per core = 8
BB = 4                   # column super-blocks (each = 2048 cols = 2 source cores)
F32 = mybir.dt.float32
F32R = mybir.dt.float32r
F8 = mybir.dt.float8e4
DR = mybir.MatmulPerfMode.DoubleRow

EXP = mybir.ActivationFunctionType.Exp
LOG = mybir.ActivationFunctionType.Ln
RELU = mybir.ActivationFunctionType.Relu
IDENT = mybir.ActivationFunctionType.Identity

NRSCOL = 3 * MT * BB + MT  # 96 real + 8 warmup scratch

_CACHE = {}


def _build():
    nc = bacc.Bacc("TRN2", target_bir_lowering=False, debug=False,
                   num_devices=NCORES)

    z1t = nc.dram_tensor("z1t", [D, R], F32R, kind="ExternalInput")
    z2t = nc.dram_tensor("z2t", [D, R], F32R, kind="ExternalInput")
    w1t = nc.dram_tensor("w1t", [D, D], F32R, kind="ExternalInput")
    w2t = nc.dram_tensor("w2t", [D, D], F32R, kind="ExternalInput")
    b1c = nc.dram_tensor("b1c", [128, KC], F32, kind="ExternalInput")
    b2c = nc.dram_tensor("b2c", [128, KC], F32, kind="ExternalInput")

    rs_out = nc.dram_tensor("rs", [128, NRSCOL], F32, kind="ExternalOutput")
    cs_out = nc.dram_tensor("colsum", [16, 512], F32, kind="ExternalOutput")
    d12_out = nc.dram_tensor("d12", [2, 512], F32, kind="ExternalOutput")

    n_all = [nc.dram_tensor(f"n_all{e}", [NCORES, KC, 128, R], F8,
                            addr_space="Shared") for e in range(2)]

    with tile.TileContext(nc) as tc:
        with tc.tile_pool(name="keep", bufs=1) as kp, \
             tc.tile_pool(name="dr", bufs=1, space="DRAM") as dr:

            # ---- persistent tiles ----
            b1s = kp.tile([128, KC], F32, name="b1s", tag="b1s")
            b2s = kp.tile([128, KC], F32, name="b2s", tag="b2s")
            nc.sync.dma_start(out=b1s, in_=b1c[:, :])
            nc.sync.dma_start(out=b2s, in_=b2c[:, :])
            ones_k = kp.tile([128, 1], F32, name="ones_k", tag="ones_k")
            nc.vector.memset(ones_k, 1.0)
            rs = kp.tile([128, NRSCOL], F32, name="rs", tag="rs")
            nc.vector.memset(rs, 0.0)
            acc = kp.tile([128, N], F32, name="acc", tag="acc")
            nc.vector.memset(acc, 0.0)
            mp = kp.tile([128, R], F32, name="mp", tag="mp")
            # fp32 normalized (for d12) and fp8 quantized (for sim matmuls),
            # layout [128, KC, R]: [p, k, r] = n[row r, dim k*128+p]
            ntf = [kp.tile([128, KC * R], F32, name=f"ntf{e}", tag=f"ntf{e}")
                   for e in range(2)]
            ntq = [kp.tile([128, KC, R], F8, name=f"ntq{e}", tag=f"ntq{e}")
                   for e in range(2)]
            n_loc = [dr.tile([KC, 128, R], F8, name=f"n_loc{e}", tag=f"n_loc{e}")
                     for e in range(2)]

            # ---- projection + normalize + quantize + gather ----
            with tc.tile_pool(name="proj", bufs=1) as pj, \
                 tc.tile_pool(name="psp", bufs=1, space="PSUM") as psp:
                w1 = [pj.tile([128, D], F32R, name=f"w1_{k}", tag=f"w1_{k}")
                      for k in range(KC)]
                w2 = [pj.tile([128, D], F32R, name=f"w2_{k}", tag=f"w2_{k}")
                      for k in range(KC)]
                for k in range(KC):
                    nc.sync.dma_start(out=w1[k], in_=w1t[k * 128:(k + 1) * 128, :])
                    nc.sync.dma_start(out=w2[k], in_=w2t[k * 128:(k + 1) * 128, :])
                ones_b = pj.tile([1, 128], F32, name="ones_b", tag="ones_b")
                nc.vector.memset(ones_b, 1.0)

                for e, zdram in enumerate((z1t, z2t)):
                    zt = [pj.tile([128, R], F32R, name=f"zt_{k}", tag=f"zt_{k}",
                                  bufs=2)
                          for k in range(KC)]
                    for k in range(KC):
                        nc.sync.dma_start(out=zt[k],
                                          in_=zdram[k * 128:(k + 1) * 128, :])

                    # layer 1 + elu
                    et = [pj.tile([128, R], F32R, name=f"et_{k}", tag=f"et_{k}",
                                  bufs=2)
                          for k in range(KC)]
                    for oc in range(KC):
                        pa = psp.tile([128, R], F32, name="pa", tag="pa", bufs=2)
                        for h in range(R // 512):
                            for k in range(KC):
                                nc.tensor.matmul(
                                    pa[:, h * 512:(h + 1) * 512],
                                    w1[k][:, oc * 128:(oc + 1) * 128],
                                    zt[k][:, h * 512:(h + 1) * 512],
                                    start=(k == 0), stop=(k == KC - 1))
                        t1 = pj.tile([128, R], F32, name="t1", tag="t1", bufs=2)
                        t2 = pj.tile([128, R], F32, name="t2", tag="t2", bufs=2)
                        nc.scalar.activation(t1, pa, EXP, bias=b1s[:, oc:oc + 1])
                        nc.vector.tensor_scalar_sub(t1, t1, 1.0)
                        nc.scalar.activation(t2, pa, RELU, bias=b1s[:, oc:oc + 1])
                        nc.vector.tensor_tensor(et[oc], t1, t2,
                                                mybir.AluOpType.min)

                    # layer 2 + bias; squared norms
                    ht = [pj.tile([128, R], F32, name=f"ht_{k}", tag=f"ht_{k}")
                          for k in range(KC)]
                    nsq = pj.tile([128, R], F32, name="nsq", tag="nsq")
                    for pc in range(KC):
                        ph = psp.tile([128, R], F32, name="pa", tag="pa", bufs=2)
                        for h in range(R // 512):
                            for k in range(KC):
                                nc.tensor.matmul(
                                    ph[:, h * 512:(h + 1) * 512],
                                    w2[k][:, pc * 128:(pc + 1) * 128],
                                    et[k][:, h * 512:(h + 1) * 512],
                                    start=(k == 0), stop=(k == KC - 1))
                        nc.scalar.activation(ht[pc], ph, IDENT,
                                             bias=b2s[:, pc:pc + 1])
                        if pc == 0:
                            nc.vector.tensor_mul(nsq, ht[pc], ht[pc])
                        else:
                            sq = pj.tile([128, R], F32, name="t1", tag="t1",
                                         bufs=2)
                            nc.vector.tensor_mul(sq, ht[pc], ht[pc])
                            nc.vector.tensor_add(nsq, nsq, sq)

                    # 1/norm via exp(-0.5*log(nsq_rowsum)), broadcast, normalize
                    nrm = psp.tile([1, R], F32, name="nrm", tag="nrm", bufs=1)
                    for h in range(R // 512):
                        nc.tensor.matmul(nrm[0:1, h * 512:(h + 1) * 512],
                                         ones_k,
                                         nsq[:, h * 512:(h + 1) * 512],
                                         start=True, stop=True)
                    sr = pj.tile([1, R], F32, name="sr", tag="sr")
                    nc.scalar.activation(sr, nrm, LOG)
                    nc.scalar.activation(sr, sr, EXP, scale=-0.5)
                    bc = psp.tile([128, R], F32, name="bc", tag="bc", bufs=1)
                    for h in range(R // 512):
                        nc.tensor.matmul(bc[:, h * 512:(h + 1) * 512],
                                         ones_b,
                                         sr[0:1, h * 512:(h + 1) * 512],
                                         start=True, stop=True)
                    for pc in range(KC):
                        nc.vector.tensor_mul(ntf[e][:, pc * R:(pc + 1) * R],
                                             ht[pc], bc)
                    # quantize to fp8 and stage for the gather
                    for k in range(KC):
                        nc.vector.tensor_copy(ntq[e][:, k, :],
                                              ntf[e][:, k * R:(k + 1) * R])
                        nc.sync.dma_start(out=n_loc[e][k], in_=ntq[e][:, k, :])
                    nc.gpsimd.collective_compute(
                        "AllGather", mybir.AluOpType.bypass,
                        replica_groups=[list(range(NCORES))],
                        ins=[n_loc[e][:].opt()],
                        outs=[n_all[e][:].opt()])

                # d12 row-dot products in fp32 (overlaps the gathers)
                m2 = pj.tile([128, R], F32, name="m2", tag="t1", bufs=2)
                nc.vector.tensor_mul(mp, ntf[0][:, 0:R], ntf[1][:, 0:R])
                for k in range(1, KC):
                    nc.vector.tensor_mul(m2, ntf[0][:, k * R:(k + 1) * R],
                                         ntf[1][:, k * R:(k + 1) * R])
                    nc.vector.tensor_add(mp, mp, m2)
                # partition-reduce d12 early (keeps PE warm during gathers);
                # reuses the nrm psum slot (free after the e=1 normalize)
                dp = psp.tile([1, R], F32, name="dp", tag="nrm", bufs=1)
                for h in range(2):
                    nc.tensor.matmul(dp[0:1, h * 512:(h + 1) * 512], ones_k,
                                     mp[:, h * 512:(h + 1) * 512],
                                     start=True, stop=True)
                    stg = pj.tile([1, 512], F32, name="stg", tag="stg", bufs=2)
                    nc.vector.tensor_copy(stg, dp[0:1, h * 512:(h + 1) * 512])
                    nc.sync.dma_start(out=d12_out[h:h + 1, :], in_=stg)

            # ---- similarity phase ----
            lq = ntq

            with tc.tile_pool(name="main", bufs=1) as mn:
              with tc.tile_pool(name="psm", bufs=1, space="PSUM") as psm:
                # local S11 warmup (no gather dep; fills the gather wait).
                # Row sums land in scratch cols 96..103 and are ignored on CPU.
                for m in range(MT):
                    pg = psm.tile([128, 2048], F32, name="pg", tag="pg", bufs=2)
                    for g in range(2):
                        for t in range(2):
                            nc.tensor.matmul(
                                pg[:, t * 512:(t + 1) * 512],
                                lq[0][:, 2 * g:2 * g + 2, m * 128:(m + 1) * 128],
                                lq[0][:, 2 * g:2 * g + 2, t * 512:(t + 1) * 512],
                                start=(g == 0), stop=(g == 1), perf_mode=DR)
                    nc.scalar.activation(pg[:, 0:1024], pg[:, 0:1024], EXP,
                                         scale=2.0,
                                         accum_out=rs[:, 96 + m:97 + m])

                def load_cols(e, bb):
                    cq3 = mn.tile([128, KC, 2048], F8, name="cq", tag="cq",
                                  bufs=2)
                    for j in range(2):
                        for k in range(KC):
                            nc.sync.dma_start(
                                out=cq3[:, k, j * 1024:(j + 1) * 1024],
                                in_=n_all[e][2 * bb + j, k])
                    return cq3

                def mm_group(pg, own3, cq3, m):
                    for g in range(2):
                        for t in range(4):
                            nc.tensor.matmul(
                                pg[:, t * 512:(t + 1) * 512],
                                own3[:, 2 * g:2 * g + 2, m * 128:(m + 1) * 128],
                                cq3[:, 2 * g:2 * g + 2, t * 512:(t + 1) * 512],
                                start=(g == 0), stop=(g == 1), perf_mode=DR)

                # phase B: S11 (rhs = gathered n1)
                for bb in range(BB):
                    cq3 = load_cols(0, bb)
                    for m in range(MT):
                        pg = psm.tile([128, 2048], F32, name="pg", tag="pg",
                                      bufs=2)
                        mm_group(pg, lq[0], cq3, m)
                        col = 0 * (MT * BB) + m * BB + bb
                        nc.scalar.activation(pg, pg, EXP, scale=2.0,
                                             accum_out=rs[:, col:col + 1])

                # phase CD: S12 + S22 (rhs = gathered n2)
                for bb in range(BB):
                    cq3 = load_cols(1, bb)
                    for m in range(MT):
                        pg = psm.tile([128, 2048], F32, name="pg", tag="pg",
                                      bufs=2)
                        mm_group(pg, lq[0], cq3, m)
                        col = 1 * (MT * BB) + m * BB + bb
                        nc.scalar.activation(pg, pg, EXP, scale=2.0,
                                             accum_out=rs[:, col:col + 1])
                        nc.vector.tensor_add(acc[:, bb * 2048:(bb + 1) * 2048],
                                             acc[:, bb * 2048:(bb + 1) * 2048],
                                             pg)

                        pg2 = psm.tile([128, 2048], F32, name="pg", tag="pg",
                                       bufs=2)
                        mm_group(pg2, lq[1], cq3, m)
                        col = 2 * (MT * BB) + m * BB + bb
                        nc.scalar.activation(pg2, pg2, EXP, scale=2.0,
                                             accum_out=rs[:, col:col + 1])

              # ---- tails: colsum partition reduction via ones-matmuls ----
              with tc.tile_pool(name="pst", bufs=1, space="PSUM") as pst:
                    for j in range(16):
                        cp = pst.tile([1, 512], F32, name="cp", tag="cp", bufs=4)
                        nc.tensor.matmul(cp, ones_k,
                                         acc[:, j * 512:(j + 1) * 512],
                                         start=True, stop=True)
                        stg = mn.tile([1, 512], F32, name="stg", tag="stg",
                                      bufs=4)
                        nc.vector.tensor_copy(stg, cp)
                        nc.sync.dma_start(out=cs_out[j:j + 1, :], in_=stg)
            nc.sync.dma_start(out=rs_out[:, :], in_=rs)

    nc.compile()
    return nc


def _get_nc():
    if "nc" not in _CACHE:
        _CACHE["nc"] = _build()
    return _CACHE["nc"]


def _round_f32r(a):
    """round to the bf16-pair representable set required by fp32r matmuls"""
    hi = a.astype(ml_dtypes.bfloat16).astype(np.float32)
    lo = (a - hi).astype(ml_dtypes.bfloat16).astype(np.float32)
    return hi + lo


def make_in_maps(pri, aux, W1, b1, W2, b2):
    pri = np.asarray(pri, dtype=np.float32)
    aux = np.asarray(aux, dtype=np.float32)
    w1t = _round_f32r(np.ascontiguousarray(np.asarray(W1, dtype=np.float32).T))
    w2t = _round_f32r(np.ascontiguousarray(np.asarray(W2, dtype=np.float32).T))
    b1 = np.asarray(b1, dtype=np.float32)
    b2 = np.asarray(b2, dtype=np.float32)
    b1c = np.ascontiguousarray(b1.reshape(KC, 128).T)
    b2c = np.ascontiguousarray(b2.reshape(KC, 128).T)
    priT = _round_f32r(np.ascontiguousarray(pri.T))
    auxT = _round_f32r(np.ascontiguousarray(aux.T))

    in_maps = []
    for c in range(NCORES):
        sl = slice(c * R, (c + 1) * R)
        in_maps.append({
            "z1t": np.ascontiguousarray(priT[:, sl]),
            "z2t": np.ascontiguousarray(auxT[:, sl]),
            "w1t": w1t, "w2t": w2t, "b1c": b1c, "b2c": b2c,
        })
    return in_maps


def assemble(results):
    """CPU assembly of the scalar loss from per-core partials"""
    E2 = np.exp(np.float64(2.0))
    colsum_full = np.zeros(N, dtype=np.float64)
    for c in range(NCORES):
        colsum_full += results[c]["colsum"].reshape(N).astype(np.float64)

    total = np.float64(0.0)
    for c in range(NCORES):
        rs = results[c]["rs"][:, :96].astype(np.float64)  # [128, 96]
        r = rs.reshape(128, 3, MT, BB).sum(-1)        # [128, 3, MT]
        # row i_local = m*128 + p  ->  transpose to [MT, 128] then flatten
        rs11 = r[:, 0, :].T.reshape(R)
        rs12 = r[:, 1, :].T.reshape(R)
        rs22 = r[:, 2, :].T.reshape(R)
        d12 = results[c]["d12"].astype(np.float64).reshape(R)
        den1 = rs11 + rs12 - E2
        den2 = rs22 + colsum_full[c * R:(c + 1) * R] - E2
        li = 0.5 * (np.log(den1) + np.log(den2)) - 2.0 * d12
        total += li.sum()

    return np.float32(total / N)


def kernel(pri_embedding, aux_embedding, W1, b1, W2, b2):
    in_maps = make_in_maps(pri_embedding, aux_embedding, W1, b1, W2, b2)
    nc = _get_nc()
    res = run_bass_kernel_spmd(nc, in_maps, list(range(NCORES))).results
    return assemble(res)
